# revision 25
# baseline (speedup 1.0000x reference)
"""Allpass biquad IIR filter (torchaudio allpass_biquad semantics) on 8 TRN2 cores.

Input x: [64, 1, 480000] f32.  y[n] = B0 x[n] + B1 x[n-1] + B2 x[n-2] - A1 y[n-1] - A2 y[n-2].

With sr=16000, f0=4000: w0 = pi/2, so cos(w0) ~ 6e-17 and B1 = A1 ~ -7e-17 —
negligible at f32 precision. Also B2 = (1+a)/(1+a) = 1.0 exactly. The
recurrence therefore splits into independent even/odd first-order streams:

    y[n] = B0 x[n] + x[n-2] - A2 y[n-2]

With w[n] = (y[n] - B0 x[n]) / q, q = 1 - A2*B0:

    w[n] = c w[n-2] + x[n-2],   c = -A2        (DVE tensor_tensor_scan,
                                                stride-2 APs for even/odd)
    y[n] = q * (w[n] + (B0/q) x[n])            (PE: two accumulating
                                                identity matmuls into PSUM;
                                                ScalarE applies exact q while
                                                casting PSUM->bf16)

Engine split per tile step: DVE runs only the two scans (serial-rate bound,
~2.5 ns/element, dtype-independent); TensorE does the combine at 1 cyc/col
(bf16); ScalarE drains PSUM; all I/O is bf16 (rel err ~3e-3, well within
tolerance) so DMA traffic is half of f32.

Sharding: pure data parallel — 8 sequences per core, each split into 16
row-segments of 30000 samples so all 128 SBUF partitions carry independent
work. |c| = 0.17: IIR memory ~16 taps, so a 32-sample host-prepared halo
(predecessor row tail, zeros at sequence starts) warms the scan state to below
tolerance (c^16 ~ 6e-13), making rows fully independent.
"""

import math

import numpy as np

# ---- fixed problem geometry ----
N_SEQ = 64
T = 480000
N_CORES = 8
SEQ_PER_CORE = N_SEQ // N_CORES  # 8
SEGS_PER_SEQ = 16
P = SEQ_PER_CORE * SEGS_PER_SEQ  # 128 partitions
SEG = T // SEGS_PER_SEQ  # 30000 samples per row
F = 6000  # columns per tile step
H = 32  # decay halo (even); c^(H/2) ~ 6e-13
HP2 = H + 2  # halo incl. 2-col lookback for x[n-2]
CH = 500  # PSUM combine chunk (<=512 f32 = one PSUM bank)


def _coeffs():
    w0 = 2.0 * math.pi * 4000.0 / 16000.0
    alpha = math.sin(w0) / (2.0 * 0.707)
    a0 = 1.0 + alpha
    b0 = np.float32((1.0 - alpha) / a0)
    a2 = np.float32((1.0 - alpha) / a0)
    c = np.float32(-float(a2))
    q = np.float32(1.0 - float(a2) * float(b0))
    return b0, c, q


def build(P=P, SEG=SEG, FS=None, H=H, CH=CH):
    """Build the per-core Bass graph (SPMD: same program on all 8 cores)."""
    import concourse.tile as tile
    from concourse import bacc, mybir

    B0f, Cf, Qf = _coeffs()
    HP2 = H + 2
    if FS is None:
        # small first tile (pipeline ramp) and small last tiles (short tail;
        # the second-to-last tile's combine overlaps the last tile's scans)
        FS = [1000, 5000, 5000, 5000, 5000, 5000, 2500, 1500]
    assert sum(FS) == SEG and all(f % CH == 0 and f % 2 == 0 for f in FS)
    assert H % 2 == 0
    FMAX = max(FS)

    bf16 = mybir.dt.bfloat16
    f32 = mybir.dt.float32

    nc = bacc.Bacc()
    x = nc.declare_dram_parameter("x", [P, SEG], bf16, isOutput=False)
    halo = nc.declare_dram_parameter("halo", [P, HP2], bf16, isOutput=False)
    # identity combine weights: w1 = I (exact), w2 = (B0/q) * I
    w1 = nc.declare_dram_parameter("w1", [128, 128], bf16, isOutput=False)
    w2 = nc.declare_dram_parameter("w2", [128, 128], bf16, isOutput=False)
    out = nc.declare_dram_parameter("out", [P, SEG], bf16, isOutput=True)

    with tile.TileContext(nc) as tc:
        with (
            tc.tile_pool(name="xp", bufs=4) as xp,
            tc.tile_pool(name="zp", bufs=2) as zp,
            tc.tile_pool(name="yp", bufs=2) as yp,
            tc.tile_pool(name="cp", bufs=1) as cp,
            tc.tile_pool(name="wp", bufs=1) as wpool,
            tc.tile_pool(name="pp", bufs=6, space="PSUM") as pp,
        ):
            # scan state coefficient (kept f32 for an exact pole)
            ctile = cp.tile([P, (H + FMAX) // 2], f32, tag="c")
            nc.gpsimd.memset(ctile[:], float(Cf))
            w1t = wpool.tile([128, 128], bf16, tag="w1")
            w2t = wpool.tile([128, 128], bf16, tag="w2")
            nc.sync.dma_start(w1t[:], w1[:])
            nc.sync.dma_start(w2t[:], w2[:])

            zprev = None
            prev_zw = None
            off = 0
            for k, F in enumerate(FS):
                u = HP2 + F if k == 0 else 2 + F
                xt = xp.tile([P, HP2 + FMAX], bf16, tag="x")
                if k == 0:
                    nc.sync.dma_start(xt[:, 0:HP2], halo[:])
                    nc.sync.dma_start(xt[:, HP2 : HP2 + F], x[:, 0:F])
                else:
                    nc.sync.dma_start(xt[:, 0:u], x[:, off - 2 : off + F])

                # w scan: even/odd phases (data1 is x itself; state fp32)
                zw = H + F if k == 0 else F
                zt = zp.tile([P, H + FMAX], bf16, tag="z")
                for ph in range(2):
                    if k == 0:
                        init = 0.0
                    else:
                        init = zprev[:, prev_zw - 2 + ph : prev_zw - 1 + ph]
                    nc.vector.tensor_tensor_scan(
                        out=zt[:, ph:zw:2],
                        data0=ctile[:, 0 : zw // 2],
                        data1=xt[:, ph:zw:2],
                        initial=init,
                        op0=mybir.AluOpType.mult,
                        op1=mybir.AluOpType.add,
                    )

                # combine y = q * (w + (B0/q) x): PE accumulates into PSUM,
                # ScalarE applies exact q while casting PSUM f32 -> bf16
                xoff = HP2 if k == 0 else 2
                zoff = H if k == 0 else 0
                yt = yp.tile([P, FMAX], bf16, tag="y")
                for j in range(F // CH):
                    pt = pp.tile([128, CH], f32, tag="ps")
                    nc.tensor.matmul(
                        pt[:],
                        w1t[:],
                        zt[:, zoff + j * CH : zoff + (j + 1) * CH],
                        start=True,
                        stop=False,
                    )
                    nc.tensor.matmul(
                        pt[:],
                        w2t[:],
                        xt[:, xoff + j * CH : xoff + (j + 1) * CH],
                        start=False,
                        stop=True,
                    )
                    nc.scalar.activation(
                        yt[:, j * CH : (j + 1) * CH],
                        pt[:],
                        mybir.ActivationFunctionType.Copy,
                        scale=float(Qf),
                    )
                nc.sync.dma_start(out[:, off : off + F], yt[:, 0:F])
                zprev = zt
                prev_zw = zw
                off += F
    nc.finalize()
    return nc


def _shard(x):
    """x: [64, 1, 480000] f32 -> list of 8 per-core input maps (bf16)."""
    import ml_dtypes

    bf16 = ml_dtypes.bfloat16
    B0f, Cf, Qf = _coeffs()
    eye = np.eye(128, dtype=bf16)
    w1 = eye
    w2 = (np.float32(float(B0f) / float(Qf)) * np.eye(128, dtype=np.float32)).astype(
        bf16
    )
    in_maps = []
    for i in range(N_CORES):
        shard = (
            np.ascontiguousarray(x[i * SEQ_PER_CORE : (i + 1) * SEQ_PER_CORE, 0, :])
            .reshape(P, SEG)
            .astype(bf16)
        )
        halo = np.zeros((P, HP2), bf16)
        halo[1:] = shard[:-1, SEG - HP2 :]
        halo[::SEGS_PER_SEQ] = 0.0  # sequence starts: rest state
        in_maps.append({"x": shard, "halo": halo, "w1": w1, "w2": w2})
    return in_maps


def _unshard(results):
    outs = [
        np.asarray(results[i]["out"]).astype(np.float32).reshape(SEQ_PER_CORE, T)
        for i in range(N_CORES)
    ]
    return np.concatenate(outs, axis=0)[:, None, :]


def _install_ntff_hook_shim():
    """This image's `antenv` lacks `axon_hooks`; register the NTFF profile
    hook module ourselves so trace=True works under axon."""
    import sys
    import types

    try:
        import antenv.axon_hooks  # noqa: F401

        return
    except ImportError:
        pass
    try:
        import antenv
        from trn_agent_boot.trn_boot import _ntff_profile_via_ctypes
    except ImportError:
        return

    state = {"hook": None}

    def set_axon_ntff_profile_hook(h):
        state["hook"] = h

    def get_axon_ntff_profile_hook():
        if state["hook"] is None:
            try:
                state["hook"] = _ntff_profile_via_ctypes("/opt/axon/libaxon_pjrt.so")
            except Exception:
                return None
        return state["hook"]

    mod = types.ModuleType("antenv.axon_hooks")
    mod.set_axon_ntff_profile_hook = set_axon_ntff_profile_hook
    mod.get_axon_ntff_profile_hook = get_axon_ntff_profile_hook
    sys.modules["antenv.axon_hooks"] = mod
    antenv.axon_hooks = mod


def run(x, trace=False):
    """Returns (y, BassKernelResults)."""
    import concourse.bass_utils as bass_utils

    _install_ntff_hook_shim()

    x = np.asarray(x)
    assert x.shape == (N_SEQ, 1, T), x.shape
    nc = build()
    res = bass_utils.run_bass_kernel_spmd(
        nc, _shard(x), core_ids=list(range(N_CORES)), trace=trace
    )
    return _unshard(res.results), res


def kernel(x):
    y, _ = run(x, trace=False)
    return y


# revision 26
# speedup vs baseline: 1.4817x; 1.4817x over previous
"""Allpass biquad IIR filter (torchaudio allpass_biquad semantics) on 8 TRN2 cores.

Input x: [64, 1, 480000] f32.  y[n] = B0 x[n] + B1 x[n-1] + B2 x[n-2] - A1 y[n-1] - A2 y[n-2].

With sr=16000, f0=4000: w0 = pi/2, so cos(w0) ~ 6e-17 and B1 = A1 ~ -7e-17 —
negligible at f32 precision. Also B2 = (1+a)/(1+a) = 1.0 exactly. The
recurrence therefore splits into independent even/odd first-order streams:

    y[n] = B0 x[n] + x[n-2] - A2 y[n-2]

With w[n] = (y[n] - B0 x[n]) / q, q = 1 - A2*B0:

    w[n] = c w[n-2] + x[n-2],   c = -A2        (DVE tensor_tensor_scan,
                                                stride-2 APs for even/odd)
    y[n] = q * (w[n] + (B0/q) x[n])            (PE: two accumulating
                                                identity matmuls into PSUM;
                                                ScalarE applies exact q while
                                                casting PSUM->bf16)

Engine split per tile step: DVE runs only the two scans (serial-rate bound,
~2.5 ns/element, dtype-independent); TensorE does the combine at 1 cyc/col
(bf16); ScalarE drains PSUM; all I/O is bf16 (rel err ~3e-3, well within
tolerance) so DMA traffic is half of f32.

Sharding: pure data parallel — 8 sequences per core, each split into 16
row-segments of 30000 samples so all 128 SBUF partitions carry independent
work. |c| = 0.17: IIR memory ~16 taps, so a 32-sample host-prepared halo
(predecessor row tail, zeros at sequence starts) warms the scan state to below
tolerance (c^16 ~ 6e-13), making rows fully independent.
"""

import math

import numpy as np

# ---- fixed problem geometry ----
N_SEQ = 64
T = 480000
N_CORES = 8
SEQ_PER_CORE = N_SEQ // N_CORES  # 8
SEGS_PER_SEQ = 16
P = SEQ_PER_CORE * SEGS_PER_SEQ  # 128 partitions
SEG = T // SEGS_PER_SEQ  # 30000 samples per row
F = 6000  # columns per tile step
H = 32  # decay halo (even); c^(H/2) ~ 6e-13
HP2 = H + 2  # halo incl. 2-col lookback for x[n-2]
CH = 500  # PSUM combine chunk (<=512 f32 = one PSUM bank)


def _coeffs():
    w0 = 2.0 * math.pi * 4000.0 / 16000.0
    alpha = math.sin(w0) / (2.0 * 0.707)
    a0 = 1.0 + alpha
    b0 = np.float32((1.0 - alpha) / a0)
    a2 = np.float32((1.0 - alpha) / a0)
    c = np.float32(-float(a2))
    q = np.float32(1.0 - float(a2) * float(b0))
    return b0, c, q


def build(P=P, SEG=SEG, FS=None, H=H, CH=CH):
    """Build the per-core Bass graph (SPMD: same program on all 8 cores)."""
    import concourse.tile as tile
    from concourse import bacc, mybir

    B0f, Cf, Qf = _coeffs()
    HP2 = H + 2
    if FS is None:
        # small first tile (pipeline ramp) and small last tiles (short tail;
        # the second-to-last tile's combine overlaps the last tile's scans)
        FS = [2000, 5000, 5000, 5000, 5000, 5000, 1500, 1500]
    assert sum(FS) == SEG and all(f % CH == 0 and f % 2 == 0 for f in FS)
    assert H % 2 == 0
    FMAX = max(FS)

    bf16 = mybir.dt.bfloat16
    f32 = mybir.dt.float32

    nc = bacc.Bacc()
    x = nc.declare_dram_parameter("x", [P, SEG], bf16, isOutput=False)
    halo = nc.declare_dram_parameter("halo", [P, HP2], bf16, isOutput=False)
    # identity combine weights: w1 = I (exact), w2 = (B0/q) * I
    w1 = nc.declare_dram_parameter("w1", [128, 128], bf16, isOutput=False)
    w2 = nc.declare_dram_parameter("w2", [128, 128], bf16, isOutput=False)
    out = nc.declare_dram_parameter("out", [P, SEG], bf16, isOutput=True)

    with tile.TileContext(nc) as tc:
        with (
            tc.tile_pool(name="xp", bufs=4) as xp,
            tc.tile_pool(name="zp", bufs=2) as zp,
            tc.tile_pool(name="yp", bufs=2) as yp,
            tc.tile_pool(name="cp", bufs=1) as cp,
            tc.tile_pool(name="wp", bufs=1) as wpool,
            tc.tile_pool(name="pp", bufs=6, space="PSUM") as pp,
        ):
            # scan state coefficient (kept f32 for an exact pole)
            ctile = cp.tile([P, (H + FMAX) // 2], f32, tag="c")
            nc.gpsimd.memset(ctile[:], float(Cf))
            w1t = wpool.tile([128, 128], bf16, tag="w1")
            w2t = wpool.tile([128, 128], bf16, tag="w2")
            nc.sync.dma_start(w1t[:], w1[:])
            nc.sync.dma_start(w2t[:], w2[:])

            zprev = None
            prev_zw = None
            off = 0
            for k, F in enumerate(FS):
                u = HP2 + F if k == 0 else 2 + F
                xt = xp.tile([P, HP2 + FMAX], bf16, tag="x")
                if k == 0:
                    nc.sync.dma_start(xt[:, 0:HP2], halo[:])
                    nc.sync.dma_start(xt[:, HP2 : HP2 + F], x[:, 0:F])
                else:
                    nc.sync.dma_start(xt[:, 0:u], x[:, off - 2 : off + F])

                # w scan: even/odd phases (data1 is x itself; state fp32)
                zw = H + F if k == 0 else F
                zt = zp.tile([P, H + FMAX], bf16, tag="z")
                for ph in range(2):
                    if k == 0:
                        init = 0.0
                    else:
                        init = zprev[:, prev_zw - 2 + ph : prev_zw - 1 + ph]
                    nc.vector.tensor_tensor_scan(
                        out=zt[:, ph:zw:2],
                        data0=ctile[:, 0 : zw // 2],
                        data1=xt[:, ph:zw:2],
                        initial=init,
                        op0=mybir.AluOpType.mult,
                        op1=mybir.AluOpType.add,
                    )

                # combine y = q * (w + (B0/q) x): PE accumulates into PSUM,
                # ScalarE applies exact q while casting PSUM f32 -> bf16
                xoff = HP2 if k == 0 else 2
                zoff = H if k == 0 else 0
                yt = yp.tile([P, FMAX], bf16, tag="y")
                for j in range(F // CH):
                    pt = pp.tile([128, CH], f32, tag="ps")
                    nc.tensor.matmul(
                        pt[:],
                        w1t[:],
                        zt[:, zoff + j * CH : zoff + (j + 1) * CH],
                        start=True,
                        stop=False,
                    )
                    nc.tensor.matmul(
                        pt[:],
                        w2t[:],
                        xt[:, xoff + j * CH : xoff + (j + 1) * CH],
                        start=False,
                        stop=True,
                    )
                    nc.scalar.activation(
                        yt[:, j * CH : (j + 1) * CH],
                        pt[:],
                        mybir.ActivationFunctionType.Copy,
                        scale=float(Qf),
                    )
                nc.sync.dma_start(out[:, off : off + F], yt[:, 0:F])
                zprev = zt
                prev_zw = zw
                off += F
    nc.finalize()
    return nc


def _shard(x):
    """x: [64, 1, 480000] f32 -> list of 8 per-core input maps (bf16)."""
    import ml_dtypes

    bf16 = ml_dtypes.bfloat16
    B0f, Cf, Qf = _coeffs()
    eye = np.eye(128, dtype=bf16)
    w1 = eye
    w2 = (np.float32(float(B0f) / float(Qf)) * np.eye(128, dtype=np.float32)).astype(
        bf16
    )
    in_maps = []
    for i in range(N_CORES):
        shard = (
            np.ascontiguousarray(x[i * SEQ_PER_CORE : (i + 1) * SEQ_PER_CORE, 0, :])
            .reshape(P, SEG)
            .astype(bf16)
        )
        halo = np.zeros((P, HP2), bf16)
        halo[1:] = shard[:-1, SEG - HP2 :]
        halo[::SEGS_PER_SEQ] = 0.0  # sequence starts: rest state
        in_maps.append({"x": shard, "halo": halo, "w1": w1, "w2": w2})
    return in_maps


def _unshard(results):
    outs = [
        np.asarray(results[i]["out"]).astype(np.float32).reshape(SEQ_PER_CORE, T)
        for i in range(N_CORES)
    ]
    return np.concatenate(outs, axis=0)[:, None, :]


def _install_ntff_hook_shim():
    """This image's `antenv` lacks `axon_hooks`; register the NTFF profile
    hook module ourselves so trace=True works under axon."""
    import sys
    import types

    try:
        import antenv.axon_hooks  # noqa: F401

        return
    except ImportError:
        pass
    try:
        import antenv
        from trn_agent_boot.trn_boot import _ntff_profile_via_ctypes
    except ImportError:
        return

    state = {"hook": None}

    def set_axon_ntff_profile_hook(h):
        state["hook"] = h

    def get_axon_ntff_profile_hook():
        if state["hook"] is None:
            try:
                state["hook"] = _ntff_profile_via_ctypes("/opt/axon/libaxon_pjrt.so")
            except Exception:
                return None
        return state["hook"]

    mod = types.ModuleType("antenv.axon_hooks")
    mod.set_axon_ntff_profile_hook = set_axon_ntff_profile_hook
    mod.get_axon_ntff_profile_hook = get_axon_ntff_profile_hook
    sys.modules["antenv.axon_hooks"] = mod
    antenv.axon_hooks = mod


def run(x, trace=False):
    """Returns (y, BassKernelResults)."""
    import concourse.bass_utils as bass_utils

    _install_ntff_hook_shim()

    x = np.asarray(x)
    assert x.shape == (N_SEQ, 1, T), x.shape
    nc = build()
    res = bass_utils.run_bass_kernel_spmd(
        nc, _shard(x), core_ids=list(range(N_CORES)), trace=trace
    )
    return _unshard(res.results), res


def kernel(x):
    y, _ = run(x, trace=False)
    return y
